# revision 1
# baseline (speedup 1.0000x reference)
"""DeepFusionCrossMamba Trainium2 kernel.

Sharding: 8 cores = (batch b in 0..3) x (direction dir in 0..1).
Odd cores (dir=1) receive time-reversed audio/video from the host and
run the identical forward program; the inter-layer fwd+bwd exchange is
orientation-aligned with per-core index data (indirect row scatter/gather)
around a pairwise AllReduce.

On-chip layout: activations feature-major [feature_chunk(128), T]; the
residual stream x is time-major [t_block(128), D] so the exchange and the
final channel LayerNorm are row-gatherable / free-dim reducible.
"""

import numpy as np

import concourse.bass as bass
import concourse.bacc as bacc
import concourse.mybir as mybir
import concourse.tile as tile
from concourse.bass import IndirectOffsetOnAxis
from concourse.bass_utils import run_bass_kernel_spmd

F32 = mybir.dt.float32
BF16 = mybir.dt.bfloat16
I32 = mybir.dt.int32
AF = mybir.ActivationFunctionType
OP = mybir.AluOpType

B, T, D = 4, 1024, 256
NM, DI, DS, DCONV, DTR = 2, 512, 16, 4, 16
NDCH = D // 128    # 2 feature chunks of d_model
NICH = DI // 128   # 4 feature chunks of d_inner
NTB = T // 128     # 8 time blocks
SP = 2             # s-values packed per scan instruction
EPS = 1e-8

_CACHE = {}


def _decl(nc, name, shape, dtype=F32, out=False):
    return nc.declare_dram_parameter(name, list(shape), dtype, isOutput=out)


def build_nc(num_cores=8, nlayers=NM, exchange=True, repeat=1):
    nc = bacc.Bacc(None, target_bir_lowering=False, debug=False)

    a_fm_d = _decl(nc, "a_fm", [D, T])
    a_tm_d = _decl(nc, "a_tm", [T, D])
    v_fm_d = _decl(nc, "v_fm", [D, T])
    w_a2v_d = _decl(nc, "w_a2v", [D, D])
    b_a2v_d = _decl(nc, "b_a2v", [128, NDCH])
    w_v2a_d = _decl(nc, "w_v2a", [D, D])
    b_v2a_d = _decl(nc, "b_v2a", [128, NDCH])
    pk_d = _decl(nc, "pk", [3, D, D])
    bn_s_d = _decl(nc, "bn_s", [128, NDCH])
    bn_b_d = _decl(nc, "bn_b", [128, NDCH])
    rmsw_d = _decl(nc, "rmsw_bc", [NM, 128, D])
    w_in_d = _decl(nc, "w_in", [NM, D, 2 * DI])
    cw_d = _decl(nc, "cw", [NM, 128, NICH * DCONV])
    cb_d = _decl(nc, "cb", [NM, 128, NICH])
    w_xp_d = _decl(nc, "w_xp", [NM, DI, 80])
    w_dt_d = _decl(nc, "w_dt", [NM, DTR, DI])
    dtb_d = _decl(nc, "dtb", [NM, 128, NICH])
    a_neg_d = _decl(nc, "a_neg", [NM, 128, NICH * DS])
    dsk_d = _decl(nc, "dsk", [NM, 128, NICH])
    w_out_d = _decl(nc, "w_out", [NM, DI, D])
    g_bc_d = _decl(nc, "g_bc", [128, D])
    be_bc_d = _decl(nc, "be_bc", [128, D])
    id128_d = _decl(nc, "id128", [128, 128])
    id128b_d = _decl(nc, "id128b", [128, 128], BF16)
    idx_d = _decl(nc, "idx", [128, NTB], I32)
    out_d = _decl(nc, "out", [T, D], out=True)

    pairs = [[i, i + 1] for i in range(0, num_cores, 2)]

    with tile.TileContext(nc) as tc:
        if repeat > 1:
            with tc.For_i(0, repeat, 1):
                _body(nc, tc, pairs, nlayers, exchange, locals())
        else:
            _body(nc, tc, pairs, nlayers, exchange, locals())
    nc.finalize()
    return nc


def _body(nc, tc, pairs, nlayers, exchange, d):
    from contextlib import ExitStack
    ctx = ExitStack()
    with ctx:
        perm = ctx.enter_context(tc.tile_pool(name="perm", bufs=1))
        pwork = ctx.enter_context(tc.tile_pool(name="pwork", bufs=1))
        pscan = ctx.enter_context(tc.tile_pool(name="pscan", bufs=1))
        phc = ctx.enter_context(tc.tile_pool(name="phc", bufs=2))
        pdiag = ctx.enter_context(tc.tile_pool(name="pdiag", bufs=2))
        psmall = ctx.enter_context(tc.tile_pool(name="psmall", bufs=2))
        ppsum = ctx.enter_context(tc.tile_pool(name="ppsum", bufs=6, space="PSUM"))
        ppsy = ctx.enter_context(tc.tile_pool(name="ppsy", bufs=2, space="PSUM"))
        pdram = ctx.enter_context(tc.tile_pool(name="pdram", bufs=1, space="DRAM"))

        th_sl = [slice(0, 512), slice(512, 1024)]

        # ---------------- persistent loads ----------------
        def load(dram, shape, name, dtype=F32, pool=perm, eng=None):
            if not isinstance(dram, bass.AP):
                dram = dram[:]
            t = pool.tile(shape, dtype, tag=name)
            (eng or nc.sync).dma_start(t[:], dram)
            return t

        a_fm = [load(d["a_fm_d"][c * 128:(c + 1) * 128, :], [128, T],
                     ["delta", "u"][c], pool=pwork) for c in range(NDCH)]
        a_tm = perm.tile([128, NTB * D], F32, tag="a_tm", name="a_tm")
        for j in range(NTB):
            nc.sync.dma_start(a_tm[:, j * D:(j + 1) * D],
                              d["a_tm_d"][j * 128:(j + 1) * 128, :])
        w_a2v = [load(d["w_a2v_d"][c * 128:(c + 1) * 128, :], [128, D], f"w_a2v{c}")
                 for c in range(NDCH)]
        w_v2a = [load(d["w_v2a_d"][c * 128:(c + 1) * 128, :], [128, D], f"w_v2a{c}")
                 for c in range(NDCH)]
        b_a2v = load(d["b_a2v_d"], [128, NDCH], "b_a2v")
        b_v2a = load(d["b_v2a_d"], [128, NDCH], "b_v2a")
        pk = [[load(d["pk_d"][k, c * 128:(c + 1) * 128, :], [128, D], f"pk{k}{c}")
               for c in range(NDCH)] for k in range(3)]
        bn_s = load(d["bn_s_d"], [128, NDCH], "bn_s")
        bn_b = load(d["bn_b_d"], [128, NDCH], "bn_b")
        g_bc = load(d["g_bc_d"], [128, D], "g_bc")
        be_bc = load(d["be_bc_d"], [128, D], "be_bc")
        id128 = load(d["id128_d"], [128, 128], "id128")
        id128b = load(d["id128b_d"], [128, 128], "id128b", BF16)
        idx = load(d["idx_d"], [128, NTB], "idx", I32)

        bounce1 = nc.dram_tensor("bounce1", [T, D], BF16)
        bounce2 = nc.dram_tensor("bounce2", [T, D], BF16)
        xdbl_dram = nc.dram_tensor("xdbl_dram", [32, T], BF16)

        # ---------------- preamble (feature-major) ----------------
        with tc.tile_pool(name="pre", bufs=1) as pre:
            g_a2v = [pwork.tile([128, T], F32, tag=f"g_z{c}", name=f"g_a2v{c}") for c in range(NDCH)]
            for ec in range(NDCH):
                for th in range(2):
                    ps = ppsum.tile([128, 512], F32, tag="ps", name="ps")
                    for dc in range(NDCH):
                        nc.tensor.matmul(ps[:], w_a2v[dc][:, ec * 128:(ec + 1) * 128],
                                         a_fm[dc][:, th_sl[th]],
                                         start=(dc == 0), stop=(dc == NDCH - 1))
                    nc.scalar.activation(g_a2v[ec][:, th_sl[th]], ps[:], AF.Sigmoid,
                                         bias=b_a2v[:, ec:ec + 1])
            # v_ref, padded for the 3-tap conv: col j holds v_ref[t=j-1]
            v_fm = [load(d["v_fm_d"][c * 128:(c + 1) * 128, :], [128, T],
                         f"xc_s{c}", pool=pwork) for c in range(NDCH)]
            v_pad = [pwork.tile([128, T + 3], F32, tag=f"xc_pad{c}", name=f"v_pad{c}") for c in range(NDCH)]
            for c in range(NDCH):
                nc.vector.memset(v_pad[c][:, 0:1], 0.0)
                nc.vector.memset(v_pad[c][:, T + 1:T + 2], 0.0)
                nc.vector.tensor_tensor(out=v_pad[c][:, 1:T + 1], in0=v_fm[c][:],
                                        in1=g_a2v[c][:], op=OP.mult)

            g_v2a = [pwork.tile([128, T], F32, tag=f"g_z{c+2}", name=f"g_v2a{c}") for c in range(NDCH)]
            for ec in range(NDCH):
                for th in range(2):
                    ps = ppsum.tile([128, 512], F32, tag="ps", name="ps")
                    for dc in range(NDCH):
                        nc.tensor.matmul(ps[:], w_v2a[dc][:, ec * 128:(ec + 1) * 128],
                                         v_pad[dc][:, th * 512 + 1: th * 512 + 513],
                                         start=(dc == 0), stop=(dc == NDCH - 1))
                    nc.scalar.activation(g_v2a[ec][:, th_sl[th]], ps[:], AF.Sigmoid,
                                         bias=b_v2a[:, ec:ec + 1])
            dlt = [pwork.tile([128, T], F32, tag=f"xc_s{c+2}", name=f"dlt{c}") for c in range(NDCH)]
            for ec in range(NDCH):
                for th in range(2):
                    ps = ppsum.tile([128, 512], F32, tag="ps", name="ps")
                    first = True
                    for k in range(3):
                        for dc in range(NDCH):
                            nc.tensor.matmul(
                                ps[:], pk[k][dc][:, ec * 128:(ec + 1) * 128],
                                v_pad[dc][:, th * 512 + k: th * 512 + k + 512],
                                start=first, stop=(k == 2 and dc == NDCH - 1))
                            first = False
                    nc.scalar.activation(dlt[ec][:, th_sl[th]], ps[:], AF.Gelu,
                                         bias=bn_b[:, ec:ec + 1],
                                         scale=bn_s[:, ec:ec + 1])
            gdlt = [pwork.tile([128, T], F32, tag=f"xc_pad{c}", name=f"gdlt{c}") for c in range(NDCH)]
            for c in range(NDCH):
                nc.gpsimd.tensor_tensor(out=gdlt[c][:], in0=g_v2a[c][:],
                                        in1=dlt[c][:], op=OP.mult)
            # x0 (time-major) = a_tm + transpose(gdlt)
            x_tm = perm.tile([128, NTB * D], F32, tag="x_tm0", name="x_tm0")
            for j in range(NTB):
                for dc in range(NDCH):
                    psT = ppsum.tile([128, 128], F32, tag="ps", name="ps")
                    nc.tensor.transpose(psT[:], gdlt[dc][:, j * 128:(j + 1) * 128],
                                        id128[:])
                    sl = slice(j * D + dc * 128, j * D + (dc + 1) * 128)
                    nc.vector.tensor_tensor(out=x_tm[:, sl], in0=a_tm[:, sl],
                                            in1=psT[:], op=OP.add)

        # ---------------- mamba layers ----------------
        for l in range(nlayers):
            rmsw_l = load(d["rmsw_d"][l], [128, D], "rmsw")
            w_in_l = [load(d["w_in_d"][l, c * 128:(c + 1) * 128, :], [128, 2 * DI],
                          f"w_in{c}", eng=nc.scalar) for c in range(NDCH)]
            cw_l = load(d["cw_d"][l], [128, NICH * DCONV], "cw")
            cb_l = load(d["cb_d"][l], [128, NICH], "cb")
            w_xp_l = [load(d["w_xp_d"][l, c * 128:(c + 1) * 128, :], [128, 80],
                           f"w_xp{c}") for c in range(NICH)]
            w_dt_l = load(d["w_dt_d"][l], [DTR, DI], "w_dt")
            dtb_l = load(d["dtb_d"][l], [128, NICH], "dtb")
            a_neg_l = load(d["a_neg_d"][l], [128, NICH * DS], "a_neg")
            dsk_l = load(d["dsk_d"][l], [128, NICH], "dsk")
            w_out_l = [load(d["w_out_d"][l, c * 128:(c + 1) * 128, :], [128, D],
                            f"w_out{c}", eng=nc.scalar) for c in range(NICH)]
            # rmsnorm stats over channel dim (free dim in TM layout)
            st = psmall.tile([128, NTB], F32, tag="st", name="st")
            sq = pwork.tile([128, D], F32, tag="sq", name="sq")
            for j in range(NTB):
                nc.scalar.activation(sq[:], x_tm[:, j * D:(j + 1) * D], AF.Square,
                                     accum_out=st[:, j:j + 1])
            ms = psmall.tile([128, NTB], F32, tag="ms", name="ms")
            nc.vector.tensor_scalar(out=ms[:], in0=st[:], scalar1=1.0 / D,
                                    scalar2=1e-5, op0=OP.mult, op1=OP.add)
            msr = psmall.tile([128, NTB], F32, tag="msr", name="msr")
            nc.vector.reciprocal(out=msr[:], in_=ms[:])
            rstd = psmall.tile([128, NTB], F32, tag="rstd", name="rstd")
            nc.scalar.activation(rstd[:], msr[:], AF.Sqrt)

            h_tm = pwork.tile([128, NTB * D], F32, tag="h_tm", name="h_tm")
            for j in range(NTB):
                nc.vector.scalar_tensor_tensor(
                    out=h_tm[:, j * D:(j + 1) * D], in0=x_tm[:, j * D:(j + 1) * D],
                    scalar=rstd[:, j:j + 1], in1=rmsw_l[:],
                    op0=OP.mult, op1=OP.mult)
            h_fm = [pwork.tile([128, T], F32, tag=f"h_fm{c}", name=f"h_fm{c}") for c in range(NDCH)]
            for j in range(NTB):
                for dc in range(NDCH):
                    psT = ppsum.tile([128, 128], F32, tag="ps", name="ps")
                    nc.tensor.transpose(
                        psT[:], h_tm[:, j * D + dc * 128: j * D + (dc + 1) * 128],
                        id128[:])
                    nc.scalar.copy(h_fm[dc][:, j * 128:(j + 1) * 128], psT[:])

            # in_proj -> xc (padded), silu(z)
            xc_pad = [pwork.tile([128, T + 3], F32, tag=f"xc_pad{c}", name=f"xc_pad{c}")
                      for c in range(NICH)]
            g_z = [pwork.tile([128, T], F32, tag=f"g_z{c}", name=f"g_z{c}") for c in range(NICH)]
            for c in range(NICH):
                nc.vector.memset(xc_pad[c][:, 0:3], 0.0)
            for ec in range(2 * NICH):
                for th in range(2):
                    ps = ppsum.tile([128, 512], F32, tag="ps", name="ps")
                    for dc in range(NDCH):
                        nc.tensor.matmul(ps[:], w_in_l[dc][:, ec * 128:(ec + 1) * 128],
                                         h_fm[dc][:, th_sl[th]],
                                         start=(dc == 0), stop=(dc == NDCH - 1))
                    if ec < NICH:
                        nc.scalar.copy(
                            xc_pad[ec][:, 3 + th * 512: 3 + th * 512 + 512],
                            ps[:])
                    else:
                        nc.scalar.activation(g_z[ec - NICH][:, th_sl[th]], ps[:],
                                             AF.Silu)
            # depthwise causal conv: DVE tensor_scalar/stt chain + silu on ACT
            xc_s = [pwork.tile([128, T], F32, tag=f"xc_s{c}", name=f"xc_s{c}") for c in range(NICH)]
            cvo = pwork.tile([128, T], F32, tag="delta1", name="cvo")
            cvp = pwork.tile([128, T], F32, tag="esp", name="cvp")
            for c in range(NICH):
                acc = [cvo, cvp]
                nc.vector.tensor_scalar(acc[0][:], xc_pad[c][:, 0:T],
                                        cw_l[:, c * DCONV:c * DCONV + 1], None,
                                        OP.mult)
                for k in range(1, DCONV):
                    nc.vector.scalar_tensor_tensor(
                        out=acc[k % 2][:], in0=xc_pad[c][:, k:k + T],
                        scalar=cw_l[:, c * DCONV + k:c * DCONV + k + 1],
                        in1=acc[(k + 1) % 2][:], op0=OP.mult, op1=OP.add)
                nc.scalar.activation(xc_s[c][:], acc[(DCONV - 1) % 2][:], AF.Silu,
                                     bias=cb_l[:, c:c + 1])
            # x_proj -> xdbl rows [dt(16) | B(16) | C(16)]
            xdbl = pwork.tile([DTR, T], F32, tag="xdbl", name="xdbl")
            xdbl_bf = pwork.tile([48, T], BF16, tag="xdbl_bf", name="xdbl_bf")
            for th in range(2):
                psx = ppsum.tile([80, 512], F32, tag="ps", name="ps")
                for c in range(NICH):
                    nc.tensor.matmul(psx[:], w_xp_l[c][:], xc_s[c][:, th_sl[th]],
                                     start=(c == 0), stop=(c == NICH - 1))
                nc.scalar.copy(xdbl[0:DTR, th_sl[th]], psx[0:DTR, :])
                nc.vector.tensor_copy(out=xdbl_bf[0:16, th_sl[th]],
                                      in_=psx[32:48, :])
                nc.vector.tensor_copy(out=xdbl_bf[32:48, th_sl[th]],
                                      in_=psx[64:80, :])
                nc.sync.dma_start(xdbl_dram[0:16, th_sl[th]],
                                  xdbl_bf[0:16, th_sl[th]])
                nc.sync.dma_start(xdbl_dram[16:32, th_sl[th]],
                                  xdbl_bf[32:48, th_sl[th]])

            # per-chunk: delta, u, scan over (s,t), y accumulation
            y_g = [pwork.tile([128, T], F32, tag=f"xc_pad{c}", name=f"y_g{c}")
                   for c in range(NICH)]
            dA2 = [pscan.tile([128, SP * T], F32, tag=f"dA{i}", name=f"dA{i}")
                   for i in range(2)]
            dBu2 = [pscan.tile([128, SP * T], BF16, tag=f"dBu{i}", name=f"dBu{i}")
                    for i in range(2)]
            hsc = pscan.tile([128, SP * T], BF16, tag="hsc", name="hsc")
            for i in range(2):
                nc.vector.memset(dA2[i][:, 0:SP * T:T], 0.0)
            for c in range(NICH):
                psd = [ppsum.tile([128, 512], F32, tag="ps", name="ps") for _ in range(2)]
                for th in range(2):
                    nc.tensor.matmul(psd[th][:], w_dt_l[:, c * 128:(c + 1) * 128],
                                     xdbl[0:DTR, th_sl[th]], start=True, stop=True)
                delta = pwork.tile([128, T], F32, tag=f"delta{c % 2}", name="delta")
                esp = pwork.tile([128, T], F32, tag="esp", name="esp")
                for th in range(2):
                    # softplus(x+b) = ln(1 + exp(x+b)); exp & ln share a table
                    nc.scalar.activation(esp[:, th_sl[th]], psd[th][:], AF.Exp,
                                         bias=dtb_l[:, c:c + 1])
                for th in range(2):
                    nc.scalar.activation(delta[:, th_sl[th]], esp[:, th_sl[th]],
                                         AF.Ln, bias=1.0)
                u = pwork.tile([128, T], BF16, tag=f"u{c % 2}", name="u")
                nc.gpsimd.tensor_tensor(out=u[:], in0=delta[:], in1=xc_s[c][:],
                                        op=OP.mult)

                psy = [ppsy.tile([128, 512], F32, tag="psy", name="psy") for _ in range(2)]
                nsp = DS // SP
                for sp in range(nsp):
                    dA = dA2[sp % 2]
                    dBu = dBu2[sp % 2]
                    for si in range(SP):
                        s = sp * SP + si
                        nc.scalar.activation(
                            dA[:, si * T + 1:(si + 1) * T], delta[:, 1:T], AF.Exp,
                            scale=a_neg_l[:, c * DS + s: c * DS + s + 1])
                    s0 = sp * SP
                    bm = phc.tile([128, SP * T], BF16, tag="bm", name="bm")
                    nc.sync.dma_start(
                        bm[:], xdbl_dram[s0:s0 + SP, :]
                        .rearrange("a b -> (a b)").partition_broadcast(128))
                    for si in range(SP):
                        nc.gpsimd.tensor_tensor(
                            out=dBu[:, si * T:(si + 1) * T], in0=u[:],
                            in1=bm[:, si * T:(si + 1) * T], op=OP.mult)
                    nc.vector.tensor_tensor_scan(
                        hsc[:], dA[:], dBu[:], 0.0, OP.mult, OP.add)
                    cm = phc.tile([128, SP * T], BF16, tag="cm", name="cm")
                    nc.sync.dma_start(
                        cm[:], xdbl_dram[16 + s0:16 + s0 + SP, :]
                        .rearrange("a b -> (a b)").partition_broadcast(128))
                    hc = phc.tile([128, SP * T], BF16, tag="hc", name="hc")
                    nc.vector.tensor_tensor(
                        out=hc[:], in0=hsc[:], in1=cm[:], op=OP.mult)
                    for si in range(SP):
                        for th in range(2):
                            nc.tensor.matmul(
                                psy[th][:], id128b[:],
                                hc[:, si * T + th * 512: si * T + th * 512 + 512],
                                start=(sp == 0 and si == 0), stop=False)
                # skip connection D_skip * xc
                dgd = pdiag.tile([128, 128], F32, tag="cdiag", name="cdiag")
                nc.vector.tensor_scalar(out=dgd[:], in0=id128[:],
                                        scalar1=dsk_l[:, c:c + 1], scalar2=None,
                                        op0=OP.mult)
                for th in range(2):
                    nc.tensor.matmul(psy[th][:], dgd[:], xc_s[c][:, th_sl[th]],
                                     start=False, stop=True)
                for th in range(2):
                    nc.vector.tensor_tensor(out=y_g[c][:, th_sl[th]], in0=psy[th][:],
                                            in1=g_z[c][:, th_sl[th]], op=OP.mult)

            # out_proj, time-major output blocks
            ytm = pwork.tile([128, NTB * D], BF16, tag="h_tm", name="ytm")
            for j in range(NTB):
                pso = ppsum.tile([128, D], F32, tag="ps", name="ps")
                for c in range(NICH):
                    nc.tensor.matmul(pso[:], y_g[c][:, j * 128:(j + 1) * 128],
                                     w_out_l[c][:],
                                     start=(c == 0), stop=(c == NICH - 1))
                nc.scalar.copy(ytm[:, j * D:(j + 1) * D], pso[:])

            # exchange: align -> AllReduce(pair) -> align back
            if exchange:
                for j in range(NTB):
                    nc.gpsimd.indirect_dma_start(
                        out=bounce1[:],
                        out_offset=IndirectOffsetOnAxis(ap=idx[:, j:j + 1], axis=0),
                        in_=ytm[:, j * D:(j + 1) * D], in_offset=None)
                nc.gpsimd.collective_compute(
                    "AllReduce", OP.add, replica_groups=pairs,
                    ins=[bounce1[:].opt()], outs=[bounce2[:].opt()])
                artm = pwork.tile([128, NTB * D], BF16, tag="h_fm0", name="artm")
                for j in range(NTB):
                    nc.gpsimd.indirect_dma_start(
                        out=artm[:, j * D:(j + 1) * D], out_offset=None,
                        in_=bounce2[:],
                        in_offset=IndirectOffsetOnAxis(ap=idx[:, j:j + 1], axis=0))
            else:
                artm = ytm
            x_new = perm.tile([128, NTB * D], F32, tag=f"x_tm{(l + 1) % 2}", name=f"x_tm{(l + 1) % 2}")
            for j in range(NTB):
                sl = slice(j * D, (j + 1) * D)
                nc.vector.tensor_tensor(out=x_new[:, sl], in0=x_tm[:, sl],
                                        in1=artm[:, sl], op=OP.add)
            x_tm = x_new

        # ---------------- final channel LayerNorm ----------------
        s_t = pwork.tile([128, NTB * D], F32, tag="h_tm", name="h_tm")
        nc.gpsimd.tensor_tensor(out=s_t[:], in0=x_tm[:], in1=a_tm[:], op=OP.add)
        stm = psmall.tile([128, NTB], F32, tag="stm", name="stm")
        stv = psmall.tile([128, NTB], F32, tag="stv", name="stv")
        dump = pwork.tile([128, D], F32, tag="sq", name="sq")
        for j in range(NTB):
            nc.scalar.activation(dump[:], s_t[:, j * D:(j + 1) * D], AF.Copy,
                                 accum_out=stm[:, j:j + 1])
            nc.scalar.activation(dump[:], s_t[:, j * D:(j + 1) * D], AF.Square,
                                 accum_out=stv[:, j:j + 1])
        mu = psmall.tile([128, NTB], F32, tag="mu", name="mu")
        nc.vector.tensor_scalar(out=mu[:], in0=stm[:], scalar1=1.0 / D, scalar2=None,
                                op0=OP.mult)
        var = psmall.tile([128, NTB], F32, tag="var", name="var")
        nc.vector.tensor_scalar(out=var[:], in0=stv[:], scalar1=1.0 / D, scalar2=None,
                                op0=OP.mult)
        mu2 = psmall.tile([128, NTB], F32, tag="mu2", name="mu2")
        nc.vector.tensor_tensor(out=mu2[:], in0=mu[:], in1=mu[:], op=OP.mult)
        nc.vector.tensor_tensor(out=var[:], in0=var[:], in1=mu2[:], op=OP.subtract)
        ve = psmall.tile([128, NTB], F32, tag="ve", name="ve")
        nc.vector.tensor_scalar(out=ve[:], in0=var[:], scalar1=EPS, scalar2=None,
                                op0=OP.add)
        vr = psmall.tile([128, NTB], F32, tag="vr", name="vr")
        nc.vector.reciprocal(out=vr[:], in_=ve[:])
        rstd2 = psmall.tile([128, NTB], F32, tag="rstd2", name="rstd2")
        nc.scalar.activation(rstd2[:], vr[:], AF.Sqrt)
        otm = pwork.tile([128, NTB * D], F32, tag="h_fm0", name="h_fm0")
        for j in range(NTB):
            sl = slice(j * D, (j + 1) * D)
            nc.vector.tensor_scalar(out=otm[:, sl], in0=s_t[:, sl],
                                    scalar1=mu[:, j:j + 1], scalar2=rstd2[:, j:j + 1],
                                    op0=OP.subtract, op1=OP.mult)
            nc.vector.tensor_tensor(out=otm[:, sl], in0=otm[:, sl], in1=g_bc[:],
                                    op=OP.mult)
            nc.vector.tensor_tensor(out=otm[:, sl], in0=otm[:, sl], in1=be_bc[:],
                                    op=OP.add)
        for j in range(NTB):
            nc.sync.dma_start(d["out_d"][j * 128:(j + 1) * 128, :],
                              otm[:, j * D:(j + 1) * D])


# ---------------- host side ----------------

def make_in_maps(inputs, num_cores=8):
    inp = {k: np.asarray(v, dtype=np.float32) for k, v in inputs.items()}
    maps = []
    for c in range(num_cores):
        b, r = c // 2, c % 2
        a = inp["audio"][b]
        v = inp["video"][b]
        if r:
            a = a[::-1]
            v = v[::-1]
        m = {}
        m["a_fm"] = np.ascontiguousarray(a.T)
        m["a_tm"] = np.ascontiguousarray(a)
        m["v_fm"] = np.ascontiguousarray(v.T)
        m["w_a2v"] = np.ascontiguousarray(inp["gate_a2v_w"].T)
        m["b_a2v"] = np.ascontiguousarray(inp["gate_a2v_b"].reshape(NDCH, 128).T)
        m["w_v2a"] = np.ascontiguousarray(inp["gate_v2a_w"].T)
        m["b_v2a"] = np.ascontiguousarray(inp["gate_v2a_b"].reshape(NDCH, 128).T)
        pk = np.stack([inp["proj_w"][:, :, (2 - k) if r else k].T for k in range(3)])
        m["pk"] = np.ascontiguousarray(pk)
        m["bn_s"] = np.ascontiguousarray(
            (inp["bn_gamma"] / np.sqrt(1.0 + 1e-5)).reshape(NDCH, 128).T)
        m["bn_b"] = np.ascontiguousarray(inp["bn_beta"].reshape(NDCH, 128).T)
        m["rmsw_bc"] = np.ascontiguousarray(
            np.broadcast_to(inp["rms_w"][:, None, :], (NM, 128, D)))
        m["w_in"] = np.ascontiguousarray(
            np.stack([inp["in_proj_w"][l, r].T for l in range(NM)]))
        m["cw"] = np.ascontiguousarray(
            inp["conv_w"][:, r].reshape(NM, NICH, 128, DCONV)
            .transpose(0, 2, 1, 3).reshape(NM, 128, NICH * DCONV))
        m["cb"] = np.ascontiguousarray(
            inp["conv_b"][:, r].reshape(NM, NICH, 128).transpose(0, 2, 1))
        w_xp_p = np.zeros((NM, DI, 80), np.float32)
        for l in range(NM):
            xp_t = inp["x_proj_w"][l, r].T  # [DI, 48]
            w_xp_p[l, :, 0:DTR] = xp_t[:, 0:DTR]
            w_xp_p[l, :, 32:32 + DS] = xp_t[:, DTR:DTR + DS]
            w_xp_p[l, :, 64:64 + DS] = xp_t[:, DTR + DS:DTR + 2 * DS]
        m["w_xp"] = w_xp_p
        m["w_dt"] = np.ascontiguousarray(
            np.stack([inp["dt_w"][l, r].T for l in range(NM)]))
        m["dtb"] = np.ascontiguousarray(
            inp["dt_b"][:, r].reshape(NM, NICH, 128).transpose(0, 2, 1))
        m["a_neg"] = np.ascontiguousarray(
            (-np.exp(inp["A_log"][:, r])).reshape(NM, NICH, 128, DS)
            .transpose(0, 2, 1, 3).reshape(NM, 128, NICH * DS))
        m["dsk"] = np.ascontiguousarray(
            inp["D_skip"][:, r].reshape(NM, NICH, 128).transpose(0, 2, 1))
        m["w_out"] = np.ascontiguousarray(
            np.stack([inp["out_w"][l, r].T for l in range(NM)]))
        m["g_bc"] = np.ascontiguousarray(
            np.broadcast_to(inp["cln_gamma"], (128, D)))
        m["be_bc"] = np.ascontiguousarray(
            np.broadcast_to(inp["cln_beta"], (128, D)))
        selm = np.zeros((80, DS * 128), np.float32)
        for s in range(DS):
            selm[32 + s, s * 128:(s + 1) * 128] = 1.0
            selm[64 + s, s * 128:(s + 1) * 128] = 1.0
        m["sel"] = selm
        m["id128"] = np.eye(128, dtype=np.float32)
        import ml_dtypes
        m["id128b"] = np.eye(128).astype(ml_dtypes.bfloat16)
        t_of = np.arange(T, dtype=np.int64).reshape(NTB, 128).T  # [128, NTB]
        if r:
            t_of = (T - 1) - t_of
        m["idx"] = np.ascontiguousarray(t_of.astype(np.int32))
        maps.append(m)
    return maps


def get_nc(num_cores=8, **kw):
    key = ("nc", num_cores, tuple(sorted(kw.items())))
    if key not in _CACHE:
        _CACHE[key] = build_nc(num_cores, **kw)
    return _CACHE[key]


def kernel(**inputs) -> np.ndarray:
    nc = get_nc(8)
    maps = make_in_maps(inputs, 8)
    res = run_bass_kernel_spmd(nc, maps, list(range(8)))
    out = np.stack([res.results[2 * b]["out"] for b in range(B)])
    return out.astype(np.float32)


if __name__ == "__main__":
    import jax
    import reference
    with jax.default_device(jax.devices("cpu")[0]):
        inputs = {k: np.asarray(v) for k, v in reference.setup_inputs().items()}
        exp = np.asarray(reference.reference(**inputs))
    got = kernel(**inputs)
    err = np.abs(got - exp).max() / (np.abs(exp).max() + 1e-12)
    print("Relative error:", err)



# revision 6
# speedup vs baseline: 3.8914x; 3.8914x over previous
"""DeepFusionCrossMamba Trainium2 kernel.

Sharding: 8 cores = (batch b in 0..3) x (direction dir in 0..1).
Odd cores (dir=1) receive time-reversed audio/video from the host and
run the identical forward program; the inter-layer fwd+bwd exchange is
orientation-aligned with per-core index data (indirect row scatter/gather)
around a pairwise AllReduce.

On-chip layout: activations feature-major [feature_chunk(128), T]; the
residual stream x is time-major [t_block(128), D] so the exchange and the
final channel LayerNorm are row-gatherable / free-dim reducible.
"""

import numpy as np

import concourse.bass as bass
import concourse.bacc as bacc
import concourse.mybir as mybir
import concourse.tile as tile
from concourse.bass import IndirectOffsetOnAxis
from concourse.bass_utils import run_bass_kernel_spmd

F32 = mybir.dt.float32
BF16 = mybir.dt.bfloat16
I32 = mybir.dt.int32
AF = mybir.ActivationFunctionType
OP = mybir.AluOpType

B, T, D = 4, 1024, 256
NM, DI, DS, DCONV, DTR = 2, 512, 16, 4, 16
NDCH = D // 128    # 2 feature chunks of d_model
NICH = DI // 128   # 4 feature chunks of d_inner
NTB = T // 128     # 8 time blocks
SP = 2             # s-values packed per scan instruction
EPS = 1e-8

_CACHE = {}

# Single-blob input packing: per-call arg-binding through the axon tunnel
# costs ~1.3 ms per bound tensor per launch, so all per-core inputs travel
# in ONE flat f32 parameter, sliced on-device. Order here defines offsets
# on both host and device. idx is int32 bit-packed (bitcast on device).
PACK = [
    ("a_fm", [D, T]),
    ("a_tm", [T, D]),
    ("v_fm", [D, T]),
    ("w_a2v", [D, D]),
    ("b_a2v", [128, NDCH]),
    ("w_v2a", [D, D]),
    ("b_v2a", [128, NDCH]),
    ("pk", [3, D, D]),
    ("bn_s", [128, NDCH]),
    ("bn_b", [128, NDCH]),
    ("rmsw_bc", [NM, 128, D]),
    ("w_in", [NM, D, 2 * DI]),
    ("cw", [NM, 128, NICH * DCONV]),
    ("cb", [NM, 128, NICH]),
    ("w_xp", [NM, DI, 80]),
    ("w_dt", [NM, DTR, DI]),
    ("dtb", [NM, 128, NICH]),
    ("a_neg", [NM, 128, NICH * DS]),
    ("dsk", [NM, 128, NICH]),
    ("w_out", [NM, DI, D]),
    ("g_bc", [128, D]),
    ("be_bc", [128, D]),
    ("id128", [128, 128]),
    ("idx", [128, NTB]),
]


def _pack_offsets():
    offs, off = {}, 0
    for name, shape in PACK:
        n = 1
        for s in shape:
            n *= s
        offs[name] = (off, list(shape))
        off += n
    return offs, off


OFFS, BLOB_N = _pack_offsets()


def _decl(nc, name, shape, dtype=F32, out=False):
    return nc.declare_dram_parameter(name, list(shape), dtype, isOutput=out)


def build_nc(num_cores=8, nlayers=NM, exchange=True, repeat=1):
    nc = bacc.Bacc(None, target_bir_lowering=False, debug=False)

    blob_d = _decl(nc, "blob", [BLOB_N])
    out_d = _decl(nc, "out", [T, D], out=True)

    pairs = [[i, i + 1] for i in range(0, num_cores, 2)]

    with tile.TileContext(nc) as tc:
        if repeat > 1:
            with tc.For_i(0, repeat, 1):
                _body(nc, tc, pairs, nlayers, exchange, locals())
        else:
            _body(nc, tc, pairs, nlayers, exchange, locals())
    nc.finalize()
    return nc


def _body(nc, tc, pairs, nlayers, exchange, d):
    from contextlib import ExitStack
    ctx = ExitStack()
    with ctx:
        perm = ctx.enter_context(tc.tile_pool(name="perm", bufs=1))
        pwork = ctx.enter_context(tc.tile_pool(name="pwork", bufs=1))
        pscan = ctx.enter_context(tc.tile_pool(name="pscan", bufs=1))
        phc = ctx.enter_context(tc.tile_pool(name="phc", bufs=2))
        pdiag = ctx.enter_context(tc.tile_pool(name="pdiag", bufs=2))
        psmall = ctx.enter_context(tc.tile_pool(name="psmall", bufs=2))
        ppsum = ctx.enter_context(tc.tile_pool(name="ppsum", bufs=6, space="PSUM"))
        ppsy = ctx.enter_context(tc.tile_pool(name="ppsy", bufs=2, space="PSUM"))
        pdram = ctx.enter_context(tc.tile_pool(name="pdram", bufs=1, space="DRAM"))

        th_sl = [slice(0, 512), slice(512, 1024)]

        # ---------------- persistent loads ----------------
        blob_ap = d["blob_d"][:]

        def bsl(name, *pre, rows=None):
            off, shape = OFFS[name]
            for i, ix in enumerate(pre):
                stride = 1
                for s in shape[i + 1:]:
                    stride *= s
                off += ix * stride
            s = shape[len(pre):]
            assert len(s) == 2
            r0, r1 = (0, s[0]) if rows is None else rows
            off += r0 * s[1]
            n = (r1 - r0) * s[1]
            return blob_ap[off:off + n].rearrange("(a b) -> a b", a=r1 - r0)

        def load(dram, shape, name, dtype=F32, pool=perm, eng=None):
            if not isinstance(dram, bass.AP):
                dram = dram[:]
            t = pool.tile(shape, dtype, tag=name)
            (eng or nc.sync).dma_start(t[:], dram)
            return t

        a_fm = [load(bsl("a_fm", rows=(c * 128, (c + 1) * 128)), [128, T],
                     ["delta", "u"][c], pool=pwork) for c in range(NDCH)]
        a_tm = perm.tile([128, NTB * D], F32, tag="a_tm", name="a_tm")
        for j in range(NTB):
            nc.sync.dma_start(a_tm[:, j * D:(j + 1) * D],
                              bsl("a_tm", rows=(j * 128, (j + 1) * 128)))
        w_a2v = [load(bsl("w_a2v", rows=(c * 128, (c + 1) * 128)), [128, D],
                      f"w_a2v{c}") for c in range(NDCH)]
        w_v2a = [load(bsl("w_v2a", rows=(c * 128, (c + 1) * 128)), [128, D],
                      f"w_v2a{c}") for c in range(NDCH)]
        b_a2v = load(bsl("b_a2v"), [128, NDCH], "b_a2v")
        b_v2a = load(bsl("b_v2a"), [128, NDCH], "b_v2a")
        pk = [[load(bsl("pk", k, rows=(c * 128, (c + 1) * 128)), [128, D],
                    f"pk{k}{c}")
               for c in range(NDCH)] for k in range(3)]
        bn_s = load(bsl("bn_s"), [128, NDCH], "bn_s")
        bn_b = load(bsl("bn_b"), [128, NDCH], "bn_b")
        g_bc = load(bsl("g_bc"), [128, D], "g_bc")
        be_bc = load(bsl("be_bc"), [128, D], "be_bc")
        id128 = load(bsl("id128"), [128, 128], "id128")
        idx = load(bsl("idx").bitcast(I32), [128, NTB], "idx", I32)
        id128b = perm.tile([128, 128], BF16, tag="id128b", name="id128b")
        nc.vector.tensor_copy(out=id128b[:], in_=id128[:])

        bounce1 = nc.dram_tensor("bounce1", [T, D], BF16)
        bounce2 = nc.dram_tensor("bounce2", [T, D], BF16)
        xdbl_dram = nc.dram_tensor("xdbl_dram", [32, T], BF16)

        # ---------------- preamble (feature-major) ----------------
        with tc.tile_pool(name="pre", bufs=1) as pre:
            g_a2v = [pwork.tile([128, T], F32, tag=f"g_z{c}", name=f"g_a2v{c}") for c in range(NDCH)]
            for ec in range(NDCH):
                for th in range(2):
                    ps = ppsum.tile([128, 512], F32, tag="ps", name="ps")
                    for dc in range(NDCH):
                        nc.tensor.matmul(ps[:], w_a2v[dc][:, ec * 128:(ec + 1) * 128],
                                         a_fm[dc][:, th_sl[th]],
                                         start=(dc == 0), stop=(dc == NDCH - 1))
                    nc.scalar.activation(g_a2v[ec][:, th_sl[th]], ps[:], AF.Sigmoid,
                                         bias=b_a2v[:, ec:ec + 1])
            # v_ref, padded for the 3-tap conv: col j holds v_ref[t=j-1]
            v_fm = [load(bsl("v_fm", rows=(c * 128, (c + 1) * 128)), [128, T],
                         f"xc_s{c}", pool=pwork) for c in range(NDCH)]
            v_pad = [pwork.tile([128, T + 3], F32, tag=f"xc_pad{c}", name=f"v_pad{c}") for c in range(NDCH)]
            for c in range(NDCH):
                nc.vector.memset(v_pad[c][:, 0:1], 0.0)
                nc.vector.memset(v_pad[c][:, T + 1:T + 2], 0.0)
                nc.vector.tensor_tensor(out=v_pad[c][:, 1:T + 1], in0=v_fm[c][:],
                                        in1=g_a2v[c][:], op=OP.mult)

            g_v2a = [pwork.tile([128, T], F32, tag=f"g_z{c+2}", name=f"g_v2a{c}") for c in range(NDCH)]
            for ec in range(NDCH):
                for th in range(2):
                    ps = ppsum.tile([128, 512], F32, tag="ps", name="ps")
                    for dc in range(NDCH):
                        nc.tensor.matmul(ps[:], w_v2a[dc][:, ec * 128:(ec + 1) * 128],
                                         v_pad[dc][:, th * 512 + 1: th * 512 + 513],
                                         start=(dc == 0), stop=(dc == NDCH - 1))
                    nc.scalar.activation(g_v2a[ec][:, th_sl[th]], ps[:], AF.Sigmoid,
                                         bias=b_v2a[:, ec:ec + 1])
            dlt = [pwork.tile([128, T], F32, tag=f"xc_s{c+2}", name=f"dlt{c}") for c in range(NDCH)]
            for ec in range(NDCH):
                for th in range(2):
                    ps = ppsum.tile([128, 512], F32, tag="ps", name="ps")
                    first = True
                    for k in range(3):
                        for dc in range(NDCH):
                            nc.tensor.matmul(
                                ps[:], pk[k][dc][:, ec * 128:(ec + 1) * 128],
                                v_pad[dc][:, th * 512 + k: th * 512 + k + 512],
                                start=first, stop=(k == 2 and dc == NDCH - 1))
                            first = False
                    nc.scalar.activation(dlt[ec][:, th_sl[th]], ps[:], AF.Gelu,
                                         bias=bn_b[:, ec:ec + 1],
                                         scale=bn_s[:, ec:ec + 1])
            gdlt = [pwork.tile([128, T], F32, tag=f"xc_pad{c}", name=f"gdlt{c}") for c in range(NDCH)]
            for c in range(NDCH):
                nc.gpsimd.tensor_tensor(out=gdlt[c][:], in0=g_v2a[c][:],
                                        in1=dlt[c][:], op=OP.mult)
            # x0 (time-major) = a_tm + transpose(gdlt)
            x_tm = perm.tile([128, NTB * D], F32, tag="x_tm0", name="x_tm0")
            for j in range(NTB):
                for dc in range(NDCH):
                    psT = ppsum.tile([128, 128], F32, tag="ps", name="ps")
                    nc.tensor.transpose(psT[:], gdlt[dc][:, j * 128:(j + 1) * 128],
                                        id128[:])
                    sl = slice(j * D + dc * 128, j * D + (dc + 1) * 128)
                    nc.vector.tensor_tensor(out=x_tm[:, sl], in0=a_tm[:, sl],
                                            in1=psT[:], op=OP.add)

        # ---------------- mamba layers ----------------
        for l in range(nlayers):
            rmsw_l = load(bsl("rmsw_bc", l), [128, D], "rmsw")
            w_in_l = [load(bsl("w_in", l, rows=(c * 128, (c + 1) * 128)),
                           [128, 2 * DI], f"w_in{c}", eng=nc.scalar)
                      for c in range(NDCH)]
            cw_l = load(bsl("cw", l), [128, NICH * DCONV], "cw")
            cb_l = load(bsl("cb", l), [128, NICH], "cb")
            w_xp_l = [load(bsl("w_xp", l, rows=(c * 128, (c + 1) * 128)),
                           [128, 80], f"w_xp{c}") for c in range(NICH)]
            w_dt_l = load(bsl("w_dt", l), [DTR, DI], "w_dt")
            dtb_l = load(bsl("dtb", l), [128, NICH], "dtb")
            a_neg_l = load(bsl("a_neg", l), [128, NICH * DS], "a_neg")
            dsk_l = load(bsl("dsk", l), [128, NICH], "dsk")
            w_out_l = [load(bsl("w_out", l, rows=(c * 128, (c + 1) * 128)),
                            [128, D], f"w_out{c}", eng=nc.scalar)
                      for c in range(NICH)]
            # rmsnorm stats over channel dim (free dim in TM layout)
            st = psmall.tile([128, NTB], F32, tag="st", name="st")
            sq = pwork.tile([128, D], F32, tag="sq", name="sq")
            for j in range(NTB):
                nc.scalar.activation(sq[:], x_tm[:, j * D:(j + 1) * D], AF.Square,
                                     accum_out=st[:, j:j + 1])
            ms = psmall.tile([128, NTB], F32, tag="ms", name="ms")
            nc.vector.tensor_scalar(out=ms[:], in0=st[:], scalar1=1.0 / D,
                                    scalar2=1e-5, op0=OP.mult, op1=OP.add)
            msr = psmall.tile([128, NTB], F32, tag="msr", name="msr")
            nc.vector.reciprocal(out=msr[:], in_=ms[:])
            rstd = psmall.tile([128, NTB], F32, tag="rstd", name="rstd")
            nc.scalar.activation(rstd[:], msr[:], AF.Sqrt)

            h_tm = pwork.tile([128, NTB * D], F32, tag="h_tm", name="h_tm")
            for j in range(NTB):
                nc.vector.scalar_tensor_tensor(
                    out=h_tm[:, j * D:(j + 1) * D], in0=x_tm[:, j * D:(j + 1) * D],
                    scalar=rstd[:, j:j + 1], in1=rmsw_l[:],
                    op0=OP.mult, op1=OP.mult)
            h_fm = [pwork.tile([128, T], F32, tag=f"h_fm{c}", name=f"h_fm{c}") for c in range(NDCH)]
            for j in range(NTB):
                for dc in range(NDCH):
                    psT = ppsum.tile([128, 128], F32, tag="ps", name="ps")
                    nc.tensor.transpose(
                        psT[:], h_tm[:, j * D + dc * 128: j * D + (dc + 1) * 128],
                        id128[:])
                    nc.scalar.copy(h_fm[dc][:, j * 128:(j + 1) * 128], psT[:])

            # in_proj -> xc (padded), silu(z)
            xc_pad = [pwork.tile([128, T + 3], F32, tag=f"xc_pad{c}", name=f"xc_pad{c}")
                      for c in range(NICH)]
            g_z = [pwork.tile([128, T], F32, tag=f"g_z{c}", name=f"g_z{c}") for c in range(NICH)]
            for c in range(NICH):
                nc.vector.memset(xc_pad[c][:, 0:3], 0.0)
            for ec in range(2 * NICH):
                for th in range(2):
                    ps = ppsum.tile([128, 512], F32, tag="ps", name="ps")
                    for dc in range(NDCH):
                        nc.tensor.matmul(ps[:], w_in_l[dc][:, ec * 128:(ec + 1) * 128],
                                         h_fm[dc][:, th_sl[th]],
                                         start=(dc == 0), stop=(dc == NDCH - 1))
                    if ec < NICH:
                        nc.scalar.copy(
                            xc_pad[ec][:, 3 + th * 512: 3 + th * 512 + 512],
                            ps[:])
                    else:
                        nc.scalar.activation(g_z[ec - NICH][:, th_sl[th]], ps[:],
                                             AF.Silu)
            # depthwise causal conv: DVE tensor_scalar/stt chain + silu on ACT
            xc_s = [pwork.tile([128, T], F32, tag=f"xc_s{c}", name=f"xc_s{c}") for c in range(NICH)]
            cvo = pwork.tile([128, T], F32, tag="delta1", name="cvo")
            cvp = pwork.tile([128, T], F32, tag="esp", name="cvp")
            for c in range(NICH):
                acc = [cvo, cvp]
                nc.vector.tensor_scalar(acc[0][:], xc_pad[c][:, 0:T],
                                        cw_l[:, c * DCONV:c * DCONV + 1], None,
                                        OP.mult)
                for k in range(1, DCONV):
                    nc.vector.scalar_tensor_tensor(
                        out=acc[k % 2][:], in0=xc_pad[c][:, k:k + T],
                        scalar=cw_l[:, c * DCONV + k:c * DCONV + k + 1],
                        in1=acc[(k + 1) % 2][:], op0=OP.mult, op1=OP.add)
                nc.scalar.activation(xc_s[c][:], acc[(DCONV - 1) % 2][:], AF.Silu,
                                     bias=cb_l[:, c:c + 1])
            # x_proj -> xdbl rows [dt(16) | B(16) | C(16)]
            xdbl = pwork.tile([DTR, T], F32, tag="xdbl", name="xdbl")
            xdbl_bf = pwork.tile([48, T], BF16, tag="xdbl_bf", name="xdbl_bf")
            for th in range(2):
                psx = ppsum.tile([80, 512], F32, tag="ps", name="ps")
                for c in range(NICH):
                    nc.tensor.matmul(psx[:], w_xp_l[c][:], xc_s[c][:, th_sl[th]],
                                     start=(c == 0), stop=(c == NICH - 1))
                nc.scalar.copy(xdbl[0:DTR, th_sl[th]], psx[0:DTR, :])
                nc.vector.tensor_copy(out=xdbl_bf[0:16, th_sl[th]],
                                      in_=psx[32:48, :])
                nc.vector.tensor_copy(out=xdbl_bf[32:48, th_sl[th]],
                                      in_=psx[64:80, :])
                nc.sync.dma_start(xdbl_dram[0:16, th_sl[th]],
                                  xdbl_bf[0:16, th_sl[th]])
                nc.sync.dma_start(xdbl_dram[16:32, th_sl[th]],
                                  xdbl_bf[32:48, th_sl[th]])

            # per-chunk: delta, u, scan over (s,t), y accumulation
            y_g = [pwork.tile([128, T], F32, tag=f"xc_pad{c}", name=f"y_g{c}")
                   for c in range(NICH)]
            dA2 = [pscan.tile([128, SP * T], F32, tag=f"dA{i}", name=f"dA{i}")
                   for i in range(2)]
            dBu2 = [pscan.tile([128, SP * T], BF16, tag=f"dBu{i}", name=f"dBu{i}")
                    for i in range(2)]
            hsc = pscan.tile([128, SP * T], BF16, tag="hsc", name="hsc")
            for i in range(2):
                nc.vector.memset(dA2[i][:, 0:SP * T:T], 0.0)
            for c in range(NICH):
                psd = [ppsum.tile([128, 512], F32, tag="ps", name="ps") for _ in range(2)]
                for th in range(2):
                    nc.tensor.matmul(psd[th][:], w_dt_l[:, c * 128:(c + 1) * 128],
                                     xdbl[0:DTR, th_sl[th]], start=True, stop=True)
                delta = pwork.tile([128, T], F32, tag=f"delta{c % 2}", name="delta")
                esp = pwork.tile([128, T], F32, tag="esp", name="esp")
                for th in range(2):
                    # softplus(x+b) = ln(1 + exp(x+b)); exp & ln share a table
                    nc.scalar.activation(esp[:, th_sl[th]], psd[th][:], AF.Exp,
                                         bias=dtb_l[:, c:c + 1])
                for th in range(2):
                    nc.scalar.activation(delta[:, th_sl[th]], esp[:, th_sl[th]],
                                         AF.Ln, bias=1.0)
                u = pwork.tile([128, T], BF16, tag=f"u{c % 2}", name="u")
                nc.gpsimd.tensor_tensor(out=u[:], in0=delta[:], in1=xc_s[c][:],
                                        op=OP.mult)

                psy = [ppsy.tile([128, 512], F32, tag="psy", name="psy") for _ in range(2)]
                nsp = DS // SP
                for sp in range(nsp):
                    dA = dA2[sp % 2]
                    dBu = dBu2[sp % 2]
                    for si in range(SP):
                        s = sp * SP + si
                        nc.scalar.activation(
                            dA[:, si * T + 1:(si + 1) * T], delta[:, 1:T], AF.Exp,
                            scale=a_neg_l[:, c * DS + s: c * DS + s + 1])
                    s0 = sp * SP
                    bm = phc.tile([128, SP * T], BF16, tag="bm", name="bm")
                    nc.sync.dma_start(
                        bm[:], xdbl_dram[s0:s0 + SP, :]
                        .rearrange("a b -> (a b)").partition_broadcast(128))
                    for si in range(SP):
                        nc.gpsimd.tensor_tensor(
                            out=dBu[:, si * T:(si + 1) * T], in0=u[:],
                            in1=bm[:, si * T:(si + 1) * T], op=OP.mult)
                    nc.vector.tensor_tensor_scan(
                        hsc[:], dA[:], dBu[:], 0.0, OP.mult, OP.add)
                    cm = phc.tile([128, SP * T], BF16, tag="cm", name="cm")
                    nc.sync.dma_start(
                        cm[:], xdbl_dram[16 + s0:16 + s0 + SP, :]
                        .rearrange("a b -> (a b)").partition_broadcast(128))
                    hc = phc.tile([128, SP * T], BF16, tag="hc", name="hc")
                    nc.vector.tensor_tensor(
                        out=hc[:], in0=hsc[:], in1=cm[:], op=OP.mult)
                    for si in range(SP):
                        for th in range(2):
                            nc.tensor.matmul(
                                psy[th][:], id128b[:],
                                hc[:, si * T + th * 512: si * T + th * 512 + 512],
                                start=(sp == 0 and si == 0), stop=False)
                # skip connection D_skip * xc
                dgd = pdiag.tile([128, 128], F32, tag="cdiag", name="cdiag")
                nc.vector.tensor_scalar(out=dgd[:], in0=id128[:],
                                        scalar1=dsk_l[:, c:c + 1], scalar2=None,
                                        op0=OP.mult)
                for th in range(2):
                    nc.tensor.matmul(psy[th][:], dgd[:], xc_s[c][:, th_sl[th]],
                                     start=False, stop=True)
                for th in range(2):
                    nc.vector.tensor_tensor(out=y_g[c][:, th_sl[th]], in0=psy[th][:],
                                            in1=g_z[c][:, th_sl[th]], op=OP.mult)

            # out_proj, time-major output blocks
            ytm = pwork.tile([128, NTB * D], BF16, tag="h_tm", name="ytm")
            for j in range(NTB):
                pso = ppsum.tile([128, D], F32, tag="ps", name="ps")
                for c in range(NICH):
                    nc.tensor.matmul(pso[:], y_g[c][:, j * 128:(j + 1) * 128],
                                     w_out_l[c][:],
                                     start=(c == 0), stop=(c == NICH - 1))
                nc.scalar.copy(ytm[:, j * D:(j + 1) * D], pso[:])

            # exchange: align -> AllReduce(pair) -> align back
            if exchange:
                for j in range(NTB):
                    nc.gpsimd.indirect_dma_start(
                        out=bounce1[:],
                        out_offset=IndirectOffsetOnAxis(ap=idx[:, j:j + 1], axis=0),
                        in_=ytm[:, j * D:(j + 1) * D], in_offset=None)
                nc.gpsimd.collective_compute(
                    "AllReduce", OP.add, replica_groups=pairs,
                    ins=[bounce1[:].opt()], outs=[bounce2[:].opt()])
                artm = pwork.tile([128, NTB * D], BF16, tag="h_fm0", name="artm")
                for j in range(NTB):
                    nc.gpsimd.indirect_dma_start(
                        out=artm[:, j * D:(j + 1) * D], out_offset=None,
                        in_=bounce2[:],
                        in_offset=IndirectOffsetOnAxis(ap=idx[:, j:j + 1], axis=0))
            else:
                artm = ytm
            x_new = perm.tile([128, NTB * D], F32, tag=f"x_tm{(l + 1) % 2}", name=f"x_tm{(l + 1) % 2}")
            for j in range(NTB):
                sl = slice(j * D, (j + 1) * D)
                nc.vector.tensor_tensor(out=x_new[:, sl], in0=x_tm[:, sl],
                                        in1=artm[:, sl], op=OP.add)
            x_tm = x_new

        # ---------------- final channel LayerNorm ----------------
        s_t = pwork.tile([128, NTB * D], F32, tag="h_tm", name="h_tm")
        nc.gpsimd.tensor_tensor(out=s_t[:], in0=x_tm[:], in1=a_tm[:], op=OP.add)
        stm = psmall.tile([128, NTB], F32, tag="stm", name="stm")
        stv = psmall.tile([128, NTB], F32, tag="stv", name="stv")
        dump = pwork.tile([128, D], F32, tag="sq", name="sq")
        for j in range(NTB):
            nc.scalar.activation(dump[:], s_t[:, j * D:(j + 1) * D], AF.Copy,
                                 accum_out=stm[:, j:j + 1])
            nc.scalar.activation(dump[:], s_t[:, j * D:(j + 1) * D], AF.Square,
                                 accum_out=stv[:, j:j + 1])
        mu = psmall.tile([128, NTB], F32, tag="mu", name="mu")
        nc.vector.tensor_scalar(out=mu[:], in0=stm[:], scalar1=1.0 / D, scalar2=None,
                                op0=OP.mult)
        var = psmall.tile([128, NTB], F32, tag="var", name="var")
        nc.vector.tensor_scalar(out=var[:], in0=stv[:], scalar1=1.0 / D, scalar2=None,
                                op0=OP.mult)
        mu2 = psmall.tile([128, NTB], F32, tag="mu2", name="mu2")
        nc.vector.tensor_tensor(out=mu2[:], in0=mu[:], in1=mu[:], op=OP.mult)
        nc.vector.tensor_tensor(out=var[:], in0=var[:], in1=mu2[:], op=OP.subtract)
        ve = psmall.tile([128, NTB], F32, tag="ve", name="ve")
        nc.vector.tensor_scalar(out=ve[:], in0=var[:], scalar1=EPS, scalar2=None,
                                op0=OP.add)
        vr = psmall.tile([128, NTB], F32, tag="vr", name="vr")
        nc.vector.reciprocal(out=vr[:], in_=ve[:])
        rstd2 = psmall.tile([128, NTB], F32, tag="rstd2", name="rstd2")
        nc.scalar.activation(rstd2[:], vr[:], AF.Sqrt)
        otm = pwork.tile([128, NTB * D], F32, tag="h_fm0", name="h_fm0")
        for j in range(NTB):
            sl = slice(j * D, (j + 1) * D)
            nc.vector.tensor_scalar(out=otm[:, sl], in0=s_t[:, sl],
                                    scalar1=mu[:, j:j + 1], scalar2=rstd2[:, j:j + 1],
                                    op0=OP.subtract, op1=OP.mult)
            nc.vector.tensor_tensor(out=otm[:, sl], in0=otm[:, sl], in1=g_bc[:],
                                    op=OP.mult)
            nc.vector.tensor_tensor(out=otm[:, sl], in0=otm[:, sl], in1=be_bc[:],
                                    op=OP.add)
        for j in range(NTB):
            nc.sync.dma_start(d["out_d"][j * 128:(j + 1) * 128, :],
                              otm[:, j * D:(j + 1) * D])


# ---------------- host side ----------------

def make_in_maps(inputs, num_cores=8):
    inp = {k: np.asarray(v, dtype=np.float32) for k, v in inputs.items()}
    maps = []
    for c in range(num_cores):
        b, r = c // 2, c % 2
        a = inp["audio"][b]
        v = inp["video"][b]
        if r:
            a = a[::-1]
            v = v[::-1]
        m = {}
        m["a_fm"] = np.ascontiguousarray(a.T)
        m["a_tm"] = np.ascontiguousarray(a)
        m["v_fm"] = np.ascontiguousarray(v.T)
        m["w_a2v"] = np.ascontiguousarray(inp["gate_a2v_w"].T)
        m["b_a2v"] = np.ascontiguousarray(inp["gate_a2v_b"].reshape(NDCH, 128).T)
        m["w_v2a"] = np.ascontiguousarray(inp["gate_v2a_w"].T)
        m["b_v2a"] = np.ascontiguousarray(inp["gate_v2a_b"].reshape(NDCH, 128).T)
        pk = np.stack([inp["proj_w"][:, :, (2 - k) if r else k].T for k in range(3)])
        m["pk"] = np.ascontiguousarray(pk)
        m["bn_s"] = np.ascontiguousarray(
            (inp["bn_gamma"] / np.sqrt(1.0 + 1e-5)).reshape(NDCH, 128).T)
        m["bn_b"] = np.ascontiguousarray(inp["bn_beta"].reshape(NDCH, 128).T)
        m["rmsw_bc"] = np.ascontiguousarray(
            np.broadcast_to(inp["rms_w"][:, None, :], (NM, 128, D)))
        m["w_in"] = np.ascontiguousarray(
            np.stack([inp["in_proj_w"][l, r].T for l in range(NM)]))
        m["cw"] = np.ascontiguousarray(
            inp["conv_w"][:, r].reshape(NM, NICH, 128, DCONV)
            .transpose(0, 2, 1, 3).reshape(NM, 128, NICH * DCONV))
        m["cb"] = np.ascontiguousarray(
            inp["conv_b"][:, r].reshape(NM, NICH, 128).transpose(0, 2, 1))
        w_xp_p = np.zeros((NM, DI, 80), np.float32)
        for l in range(NM):
            xp_t = inp["x_proj_w"][l, r].T  # [DI, 48]
            w_xp_p[l, :, 0:DTR] = xp_t[:, 0:DTR]
            w_xp_p[l, :, 32:32 + DS] = xp_t[:, DTR:DTR + DS]
            w_xp_p[l, :, 64:64 + DS] = xp_t[:, DTR + DS:DTR + 2 * DS]
        m["w_xp"] = w_xp_p
        m["w_dt"] = np.ascontiguousarray(
            np.stack([inp["dt_w"][l, r].T for l in range(NM)]))
        m["dtb"] = np.ascontiguousarray(
            inp["dt_b"][:, r].reshape(NM, NICH, 128).transpose(0, 2, 1))
        m["a_neg"] = np.ascontiguousarray(
            (-np.exp(inp["A_log"][:, r])).reshape(NM, NICH, 128, DS)
            .transpose(0, 2, 1, 3).reshape(NM, 128, NICH * DS))
        m["dsk"] = np.ascontiguousarray(
            inp["D_skip"][:, r].reshape(NM, NICH, 128).transpose(0, 2, 1))
        m["w_out"] = np.ascontiguousarray(
            np.stack([inp["out_w"][l, r].T for l in range(NM)]))
        m["g_bc"] = np.ascontiguousarray(
            np.broadcast_to(inp["cln_gamma"], (128, D)))
        m["be_bc"] = np.ascontiguousarray(
            np.broadcast_to(inp["cln_beta"], (128, D)))
        m["id128"] = np.eye(128, dtype=np.float32)
        t_of = np.arange(T, dtype=np.int64).reshape(NTB, 128).T  # [128, NTB]
        if r:
            t_of = (T - 1) - t_of
        m["idx"] = np.ascontiguousarray(t_of.astype(np.int32))

        parts = []
        for name, shape in PACK:
            a = np.ascontiguousarray(m[name])
            assert a.shape == tuple(shape), (name, a.shape, shape)
            if a.dtype == np.int32:
                a = a.view(np.float32)
            parts.append(a.astype(np.float32, copy=False).ravel())
        maps.append({"blob": np.concatenate(parts)})
    return maps


def get_nc(num_cores=8, **kw):
    key = ("nc", num_cores, tuple(sorted(kw.items())))
    if key not in _CACHE:
        _CACHE[key] = build_nc(num_cores, **kw)
    return _CACHE[key]


def kernel(**inputs) -> np.ndarray:
    nc = get_nc(8)
    maps = make_in_maps(inputs, 8)
    res = run_bass_kernel_spmd(nc, maps, list(range(8)))
    out = np.stack([res.results[2 * b]["out"] for b in range(B)])
    return out.astype(np.float32)


if __name__ == "__main__":
    import jax
    import reference
    with jax.default_device(jax.devices("cpu")[0]):
        inputs = {k: np.asarray(v) for k, v in reference.setup_inputs().items()}
        exp = np.asarray(reference.reference(**inputs))
    got = kernel(**inputs)
    err = np.abs(got - exp).max() / (np.abs(exp).max() + 1e-12)
    print("Relative error:", err)



# revision 14
# speedup vs baseline: 4.0337x; 1.0366x over previous
"""DeepFusionCrossMamba Trainium2 kernel.

Launch-overhead-first design: through the axon tunnel each bound tensor
costs ~1.3 ms per call and each extra core in the mesh adds dispatch
bookkeeping, while the whole network is only ~3 ms of device work. So a
single core runs all 4 batches with fwd+bwd merged locally (no
collectives), and all inputs travel in ONE flat f32 blob parameter.

On-chip layout per batch: activations feature-major [feature(128), T];
the residual stream x is time-major [t_block(128), D]. The bwd direction
consumes a time-reversed copy of h made by a DRAM bounce + indirect row
gather (idx holds the reversed row permutation); its output is reversed
back the same way before the residual add.
"""

import numpy as np

import concourse.bass as bass
import concourse.bacc as bacc
import concourse.mybir as mybir
import concourse.tile as tile
from concourse.bass import IndirectOffsetOnAxis
from concourse.bass_utils import run_bass_kernel_spmd

F32 = mybir.dt.float32
BF16 = mybir.dt.bfloat16
I32 = mybir.dt.int32
AF = mybir.ActivationFunctionType
OP = mybir.AluOpType

B, T, D = 4, 1024, 256
NM, DI, DS, DCONV, DTR = 2, 512, 16, 4, 16
NDCH = D // 128    # 2 feature chunks of d_model
NICH = DI // 128   # 4 feature chunks of d_inner
NTB = T // 128     # 8 time blocks
SP = 2             # s-values packed per scan instruction
EPS = 1e-8
NBPC = 4           # batches per core (all of them; single-core mesh)

_CACHE = {}

# Single-blob input packing. Order defines offsets on host and device.
# idx is int32 bit-packed (bitcast on device).
PACK = [
    ("a_fm", [NBPC, D, T]),
    ("a_tm", [NBPC, T, D]),
    ("v_fm", [NBPC, D, T]),
    ("w_a2v", [D, D]),
    ("b_a2v", [128, NDCH]),
    ("w_v2a", [D, D]),
    ("b_v2a", [128, NDCH]),
    ("pk", [3, D, D]),
    ("bn_s", [128, NDCH]),
    ("bn_b", [128, NDCH]),
    ("rmsw_bc", [NM, 128, D]),
    ("w_in", [NM, 2, D, 2 * DI]),
    ("cw", [NM, 2, 128, NICH * DCONV]),
    ("cb", [NM, 2, 128, NICH]),
    ("w_xp", [NM, 2, DI, 80]),
    ("w_dt", [NM, 2, DTR, DI]),
    ("dtb", [NM, 2, 128, NICH]),
    ("a_neg", [NM, 2, 128, NICH * DS]),
    ("dsk", [NM, 2, 128, NICH]),
    ("w_out", [NM, 2, DI, D]),
    ("g_bc", [128, D]),
    ("be_bc", [128, D]),
    ("id128", [128, 128]),
    ("idx", [128, NTB]),
]


def _pack_offsets():
    offs, off = {}, 0
    for name, shape in PACK:
        n = 1
        for s in shape:
            n *= s
        offs[name] = (off, list(shape))
        off += n
    return offs, off


OFFS, BLOB_N = _pack_offsets()


def _decl(nc, name, shape, dtype=F32, out=False):
    return nc.declare_dram_parameter(name, list(shape), dtype, isOutput=out)


def build_nc(nlayers=NM, nbatches=NBPC):
    nc = bacc.Bacc(None, target_bir_lowering=False, debug=False)
    blob_d = _decl(nc, "blob", [BLOB_N])
    out_d = _decl(nc, "out", [NBPC * T, D], out=True)
    with tile.TileContext(nc) as tc:
        _body(nc, tc, nlayers, nbatches, blob_d, out_d)
    nc.finalize()
    return nc


def _body(nc, tc, nlayers, nbatches, blob_d, out_d):
    from contextlib import ExitStack
    ctx = ExitStack()
    with ctx:
        perm = ctx.enter_context(tc.tile_pool(name="perm", bufs=1))
        pwork = ctx.enter_context(tc.tile_pool(name="pwork", bufs=1))
        pscan = ctx.enter_context(tc.tile_pool(name="pscan", bufs=1))
        phc = ctx.enter_context(tc.tile_pool(name="phc", bufs=2))
        pdiag = ctx.enter_context(tc.tile_pool(name="pdiag", bufs=2))
        psmall = ctx.enter_context(tc.tile_pool(name="psmall", bufs=2))
        ppsum = ctx.enter_context(tc.tile_pool(name="ppsum", bufs=6, space="PSUM"))
        ppsy = ctx.enter_context(tc.tile_pool(name="ppsy", bufs=2, space="PSUM"))

        th_sl = [slice(0, 512), slice(512, 1024)]

        blob_ap = blob_d[:]

        def bsl(name, *pre, rows=None):
            off, shape = OFFS[name]
            for i, ix in enumerate(pre):
                stride = 1
                for s in shape[i + 1:]:
                    stride *= s
                off += ix * stride
            s = shape[len(pre):]
            assert len(s) == 2
            r0, r1 = (0, s[0]) if rows is None else rows
            off += r0 * s[1]
            n = (r1 - r0) * s[1]
            return blob_ap[off:off + n].rearrange("(a b) -> a b", a=r1 - r0)

        def load(dram, shape, name, dtype=F32, pool=perm, eng=None):
            if not isinstance(dram, bass.AP):
                dram = dram[:]
            t = pool.tile(shape, dtype, tag=name)
            (eng or nc.sync).dma_start(t[:], dram)
            return t

        # ---------------- shared persistent loads ----------------
        w_a2v = [load(bsl("w_a2v", rows=(c * 128, (c + 1) * 128)), [128, D],
                      f"w_a2v{c}") for c in range(NDCH)]
        w_v2a = [load(bsl("w_v2a", rows=(c * 128, (c + 1) * 128)), [128, D],
                      f"w_v2a{c}") for c in range(NDCH)]
        b_a2v = load(bsl("b_a2v"), [128, NDCH], "b_a2v")
        b_v2a = load(bsl("b_v2a"), [128, NDCH], "b_v2a")
        pk = [[load(bsl("pk", k, rows=(c * 128, (c + 1) * 128)), [128, D],
                    f"pk{k}{c}")
               for c in range(NDCH)] for k in range(3)]
        bn_s = load(bsl("bn_s"), [128, NDCH], "bn_s")
        bn_b = load(bsl("bn_b"), [128, NDCH], "bn_b")
        g_bc = load(bsl("g_bc"), [128, D], "g_bc")
        be_bc = load(bsl("be_bc"), [128, D], "be_bc")
        id128 = load(bsl("id128"), [128, 128], "id128")
        idx = load(bsl("idx").bitcast(I32), [128, NTB], "idx", I32)
        id128b = perm.tile([128, 128], BF16, tag="id128b", name="id128b")
        nc.vector.tensor_copy(out=id128b[:], in_=id128[:])
        rmsw = [load(bsl("rmsw_bc", l), [128, D], f"rmsw{l}")
                for l in range(nlayers)]

        bounceY = [nc.dram_tensor(f"bounceY{i}", [T, D], BF16) for i in range(2)]
        xdbl_dram = nc.dram_tensor("xdbl_dram", [32, T], BF16)

        for bi in range(nbatches):
            # ---------------- preamble (feature-major) ----------------
            a_fm = [load(bsl("a_fm", bi, rows=(c * 128, (c + 1) * 128)), [128, T],
                         ["delta0", "delta1"][c], pool=pwork) for c in range(NDCH)]
            a_tm = perm.tile([128, NTB * D], F32, tag="a_tm", name="a_tm")
            for j in range(NTB):
                nc.sync.dma_start(a_tm[:, j * D:(j + 1) * D],
                                  bsl("a_tm", bi, rows=(j * 128, (j + 1) * 128)))
            g_a2v = [pwork.tile([128, T], F32, tag=f"g_z{c}", name=f"g_a2v{c}")
                     for c in range(NDCH)]
            for ec in range(NDCH):
                for th in range(2):
                    ps = ppsum.tile([128, 512], F32, tag="ps", name="ps")
                    for dc in range(NDCH):
                        nc.tensor.matmul(ps[:], w_a2v[dc][:, ec * 128:(ec + 1) * 128],
                                         a_fm[dc][:, th_sl[th]],
                                         start=(dc == 0), stop=(dc == NDCH - 1))
                    nc.scalar.activation(g_a2v[ec][:, th_sl[th]], ps[:], AF.Sigmoid,
                                         bias=b_a2v[:, ec:ec + 1])
            # v_ref, padded for the 3-tap conv: col j holds v_ref[t=j-1]
            v_fm = [load(bsl("v_fm", bi, rows=(c * 128, (c + 1) * 128)), [128, T],
                         f"xc_s{c}", pool=pwork) for c in range(NDCH)]
            v_pad = [pwork.tile([128, T + 3], F32, tag=f"xc_pad{c}", name=f"v_pad{c}")
                     for c in range(NDCH)]
            for c in range(NDCH):
                nc.vector.memset(v_pad[c][:, 0:1], 0.0)
                nc.vector.memset(v_pad[c][:, T + 1:T + 2], 0.0)
                nc.vector.tensor_tensor(out=v_pad[c][:, 1:T + 1], in0=v_fm[c][:],
                                        in1=g_a2v[c][:], op=OP.mult)

            g_v2a = [pwork.tile([128, T], F32, tag=f"g_z{c+2}", name=f"g_v2a{c}")
                     for c in range(NDCH)]
            for ec in range(NDCH):
                for th in range(2):
                    ps = ppsum.tile([128, 512], F32, tag="ps", name="ps")
                    for dc in range(NDCH):
                        nc.tensor.matmul(ps[:], w_v2a[dc][:, ec * 128:(ec + 1) * 128],
                                         v_pad[dc][:, th * 512 + 1: th * 512 + 513],
                                         start=(dc == 0), stop=(dc == NDCH - 1))
                    nc.scalar.activation(g_v2a[ec][:, th_sl[th]], ps[:], AF.Sigmoid,
                                         bias=b_v2a[:, ec:ec + 1])
            dlt = [pwork.tile([128, T], F32, tag=f"xc_s{c+2}", name=f"dlt{c}")
                   for c in range(NDCH)]
            for ec in range(NDCH):
                for th in range(2):
                    ps = ppsum.tile([128, 512], F32, tag="ps", name="ps")
                    first = True
                    for k in range(3):
                        for dc in range(NDCH):
                            nc.tensor.matmul(
                                ps[:], pk[k][dc][:, ec * 128:(ec + 1) * 128],
                                v_pad[dc][:, th * 512 + k: th * 512 + k + 512],
                                start=first, stop=(k == 2 and dc == NDCH - 1))
                            first = False
                    nc.scalar.activation(dlt[ec][:, th_sl[th]], ps[:], AF.Gelu,
                                         bias=bn_b[:, ec:ec + 1],
                                         scale=bn_s[:, ec:ec + 1])
            gdlt = [pwork.tile([128, T], F32, tag=f"xc_pad{c}", name=f"gdlt{c}")
                    for c in range(NDCH)]
            for c in range(NDCH):
                nc.gpsimd.tensor_tensor(out=gdlt[c][:], in0=g_v2a[c][:],
                                        in1=dlt[c][:], op=OP.mult)
            # x0 (time-major) = a_tm + transpose(gdlt)
            x_tm = perm.tile([128, NTB * D], F32, tag="x_tm0", name="x_tm0")
            for j in range(NTB):
                for dc in range(NDCH):
                    psT = ppsum.tile([128, 128], F32, tag="ps", name="ps")
                    nc.tensor.transpose(psT[:], gdlt[dc][:, j * 128:(j + 1) * 128],
                                        id128[:])
                    sl = slice(j * D + dc * 128, j * D + (dc + 1) * 128)
                    nc.vector.tensor_tensor(out=x_tm[:, sl], in0=a_tm[:, sl],
                                            in1=psT[:], op=OP.add)

            # ---------------- mamba layers ----------------
            for l in range(nlayers):
                # rmsnorm stats over channel dim (free dim in TM layout)
                st = psmall.tile([128, NTB], F32, tag="st", name="st")
                sq = pwork.tile([128, D], F32, tag="sq", name="sq")
                for j in range(NTB):
                    nc.scalar.activation(sq[:], x_tm[:, j * D:(j + 1) * D],
                                         AF.Square, accum_out=st[:, j:j + 1])
                ms = psmall.tile([128, NTB], F32, tag="ms", name="ms")
                nc.vector.tensor_scalar(out=ms[:], in0=st[:], scalar1=1.0 / D,
                                        scalar2=1e-5, op0=OP.mult, op1=OP.add)
                msr = psmall.tile([128, NTB], F32, tag="msr", name="msr")
                nc.vector.reciprocal(out=msr[:], in_=ms[:])
                rstd = psmall.tile([128, NTB], F32, tag="rstd", name="rstd")
                nc.scalar.activation(rstd[:], msr[:], AF.Sqrt)

                h_tm = pwork.tile([128, NTB * D], F32, tag="h_tm", name="h_tm")
                for j in range(NTB):
                    nc.vector.scalar_tensor_tensor(
                        out=h_tm[:, j * D:(j + 1) * D],
                        in0=x_tm[:, j * D:(j + 1) * D],
                        scalar=rstd[:, j:j + 1], in1=rmsw[l][:],
                        op0=OP.mult, op1=OP.mult)
                h_fm = [pwork.tile([128, T], F32, tag=f"h_fm{c}", name=f"h_fm{c}")
                        for c in range(NDCH)]
                for j in range(NTB):
                    for dc in range(NDCH):
                        psT = ppsum.tile([128, 128], F32, tag="ps", name="ps")
                        nc.tensor.transpose(
                            psT[:], h_tm[:, j * D + dc * 128: j * D + (dc + 1) * 128],
                            id128[:])
                        nc.scalar.copy(h_fm[dc][:, j * 128:(j + 1) * 128], psT[:])
                # bwd branch consumes h with the time (free) dim reversed —
                # the PE moving operand reads a negative-stride AP directly
                h_rev_fm = [h_fm[c][:, ::-1] for c in range(NDCH)]

                ytm = [None, None]
                for r in range(2):
                    hsrc = h_fm if r == 0 else h_rev_fm
                    w_in_l = [load(bsl("w_in", l, r, rows=(c * 128, (c + 1) * 128)),
                                   [128, 2 * DI], f"w_in{c}", eng=nc.scalar)
                              for c in range(NDCH)]
                    cw_l = load(bsl("cw", l, r), [128, NICH * DCONV], "cw")
                    cb_l = load(bsl("cb", l, r), [128, NICH], "cb")
                    w_xp_l = [load(bsl("w_xp", l, r, rows=(c * 128, (c + 1) * 128)),
                                   [128, 80], f"w_xp{c}") for c in range(NICH)]
                    w_dt_l = load(bsl("w_dt", l, r), [DTR, DI], "w_dt")
                    dtb_l = load(bsl("dtb", l, r), [128, NICH], "dtb")
                    a_neg_l = load(bsl("a_neg", l, r), [128, NICH * DS], "a_neg")
                    dsk_l = load(bsl("dsk", l, r), [128, NICH], "dsk")
                    w_out_l = [load(bsl("w_out", l, r, rows=(c * 128, (c + 1) * 128)),
                                    [128, D], f"w_out{c}", eng=nc.scalar)
                               for c in range(NICH)]

                    # in_proj -> xc (padded), silu(z)
                    xc_pad = [pwork.tile([128, T + 3], F32, tag=f"xc_pad{c}",
                                         name=f"xc_pad{c}") for c in range(NICH)]
                    g_z = [pwork.tile([128, T], F32, tag=f"g_z{c}", name=f"g_z{c}")
                           for c in range(NICH)]
                    for c in range(NICH):
                        nc.vector.memset(xc_pad[c][:, 0:3], 0.0)
                    for ec in range(2 * NICH):
                        for th in range(2):
                            ps = ppsum.tile([128, 512], F32, tag="ps", name="ps")
                            for dc in range(NDCH):
                                nc.tensor.matmul(
                                    ps[:], w_in_l[dc][:, ec * 128:(ec + 1) * 128],
                                    hsrc[dc][:, th_sl[th]],
                                    start=(dc == 0), stop=(dc == NDCH - 1))
                            if ec < NICH:
                                nc.scalar.copy(
                                    xc_pad[ec][:, 3 + th * 512: 3 + th * 512 + 512],
                                    ps[:])
                            else:
                                nc.scalar.activation(g_z[ec - NICH][:, th_sl[th]],
                                                     ps[:], AF.Silu)
                    # depthwise causal conv: DVE tensor_scalar/stt chain + silu
                    xc_s = [pwork.tile([128, T], F32, tag=f"xc_s{c}",
                                       name=f"xc_s{c}") for c in range(NICH)]
                    cvo = pwork.tile([128, T], F32, tag="delta1", name="cvo")
                    cvp = pwork.tile([128, T], F32, tag="esp", name="cvp")
                    for c in range(NICH):
                        acc = [cvo, cvp]
                        nc.vector.tensor_scalar(acc[0][:], xc_pad[c][:, 0:T],
                                                cw_l[:, c * DCONV:c * DCONV + 1],
                                                None, OP.mult)
                        for k in range(1, DCONV):
                            nc.vector.scalar_tensor_tensor(
                                out=acc[k % 2][:], in0=xc_pad[c][:, k:k + T],
                                scalar=cw_l[:, c * DCONV + k:c * DCONV + k + 1],
                                in1=acc[(k + 1) % 2][:], op0=OP.mult, op1=OP.add)
                        nc.scalar.activation(xc_s[c][:], acc[(DCONV - 1) % 2][:],
                                             AF.Silu, bias=cb_l[:, c:c + 1])
                    # x_proj -> xdbl rows [dt(16) | B(16) | C(16)]
                    xdbl = pwork.tile([DTR, T], F32, tag="xdbl", name="xdbl")
                    xdbl_bf = pwork.tile([48, T], BF16, tag="xdbl_bf", name="xdbl_bf")
                    for th in range(2):
                        psx = ppsum.tile([80, 512], F32, tag="ps", name="ps")
                        for c in range(NICH):
                            nc.tensor.matmul(psx[:], w_xp_l[c][:],
                                             xc_s[c][:, th_sl[th]],
                                             start=(c == 0), stop=(c == NICH - 1))
                        nc.scalar.copy(xdbl[0:DTR, th_sl[th]], psx[0:DTR, :])
                        nc.vector.tensor_copy(out=xdbl_bf[0:16, th_sl[th]],
                                              in_=psx[32:48, :])
                        nc.vector.tensor_copy(out=xdbl_bf[32:48, th_sl[th]],
                                              in_=psx[64:80, :])
                        nc.sync.dma_start(xdbl_dram[0:16, th_sl[th]],
                                          xdbl_bf[0:16, th_sl[th]])
                        nc.sync.dma_start(xdbl_dram[16:32, th_sl[th]],
                                          xdbl_bf[32:48, th_sl[th]])

                    # per-chunk: delta, u, scan over (s,t), y accumulation
                    y_g = [pwork.tile([128, T], F32, tag=f"xc_pad{c}",
                                      name=f"y_g{c}") for c in range(NICH)]
                    dA2 = [pscan.tile([128, SP * T], F32, tag=f"dA{i}",
                                      name=f"dA{i}") for i in range(2)]
                    dBu2 = [pscan.tile([128, SP * T], BF16, tag=f"dBu{i}",
                                       name=f"dBu{i}") for i in range(2)]
                    hsc = pscan.tile([128, SP * T], BF16, tag="hsc", name="hsc")
                    for i in range(2):
                        nc.vector.memset(dA2[i][:, 0:SP * T:T], 0.0)
                    for c in range(NICH):
                        psd = [ppsum.tile([128, 512], F32, tag="ps", name="ps")
                               for _ in range(2)]
                        for th in range(2):
                            nc.tensor.matmul(psd[th][:],
                                             w_dt_l[:, c * 128:(c + 1) * 128],
                                             xdbl[0:DTR, th_sl[th]],
                                             start=True, stop=True)
                        delta = pwork.tile([128, T], F32, tag=f"delta{c % 2}",
                                           name="delta")
                        esp = pwork.tile([128, T], F32, tag="esp", name="esp")
                        for th in range(2):
                            # softplus(x+b) = ln(1+exp(x+b)); exp & ln share a table
                            nc.scalar.activation(esp[:, th_sl[th]], psd[th][:],
                                                 AF.Exp, bias=dtb_l[:, c:c + 1])
                        for th in range(2):
                            nc.scalar.activation(delta[:, th_sl[th]],
                                                 esp[:, th_sl[th]], AF.Ln, bias=1.0)
                        u = pwork.tile([128, T], BF16, tag=f"u{c % 2}", name="u")
                        nc.gpsimd.tensor_tensor(out=u[:], in0=delta[:],
                                                in1=xc_s[c][:], op=OP.mult)

                        psy = [ppsy.tile([128, 512], F32, tag="psy", name="psy")
                               for _ in range(2)]
                        nsp = DS // SP
                        for sp in range(nsp):
                            dA = dA2[sp % 2]
                            dBu = dBu2[sp % 2]
                            for si in range(SP):
                                s = sp * SP + si
                                nc.scalar.activation(
                                    dA[:, si * T + 1:(si + 1) * T], delta[:, 1:T],
                                    AF.Exp,
                                    scale=a_neg_l[:, c * DS + s: c * DS + s + 1])
                            s0 = sp * SP
                            bm = phc.tile([128, SP * T], BF16, tag="bm", name="bm")
                            nc.sync.dma_start(
                                bm[:], xdbl_dram[s0:s0 + SP, :]
                                .rearrange("a b -> (a b)").partition_broadcast(128))
                            for si in range(SP):
                                nc.gpsimd.tensor_tensor(
                                    out=dBu[:, si * T:(si + 1) * T], in0=u[:],
                                    in1=bm[:, si * T:(si + 1) * T], op=OP.mult)
                            nc.vector.tensor_tensor_scan(
                                hsc[:], dA[:], dBu[:], 0.0, OP.mult, OP.add)
                            cm = phc.tile([128, SP * T], BF16, tag="cm", name="cm")
                            nc.sync.dma_start(
                                cm[:], xdbl_dram[16 + s0:16 + s0 + SP, :]
                                .rearrange("a b -> (a b)").partition_broadcast(128))
                            hc = phc.tile([128, SP * T], BF16, tag="hc", name="hc")
                            nc.vector.tensor_tensor(
                                out=hc[:], in0=hsc[:], in1=cm[:], op=OP.mult)
                            for si in range(SP):
                                for th in range(2):
                                    nc.tensor.matmul(
                                        psy[th][:], id128b[:],
                                        hc[:, si * T + th * 512: si * T + th * 512 + 512],
                                        start=(sp == 0 and si == 0), stop=False)
                        # skip connection D_skip * xc
                        dgd = pdiag.tile([128, 128], F32, tag="cdiag", name="cdiag")
                        nc.vector.tensor_scalar(out=dgd[:], in0=id128[:],
                                                scalar1=dsk_l[:, c:c + 1],
                                                scalar2=None, op0=OP.mult)
                        for th in range(2):
                            nc.tensor.matmul(psy[th][:], dgd[:],
                                             xc_s[c][:, th_sl[th]],
                                             start=False, stop=True)
                        for th in range(2):
                            nc.vector.tensor_tensor(out=y_g[c][:, th_sl[th]],
                                                    in0=psy[th][:],
                                                    in1=g_z[c][:, th_sl[th]],
                                                    op=OP.mult)

                    # out_proj, time-major output blocks
                    ytag = "h_tm" if r == 0 else "h_rev"
                    ytm[r] = pwork.tile([128, NTB * D], BF16, tag=ytag,
                                        name=f"ytm{r}")
                    for j in range(NTB):
                        pso = ppsum.tile([128, D], F32, tag="ps", name="ps")
                        for c in range(NICH):
                            nc.tensor.matmul(pso[:], y_g[c][:, j * 128:(j + 1) * 128],
                                             w_out_l[c][:],
                                             start=(c == 0), stop=(c == NICH - 1))
                        nc.scalar.copy(ytm[r][:, j * D:(j + 1) * D], pso[:])

                # reverse bwd output back to natural frame via bounce
                by = bounceY[l % 2]
                for j in range(NTB):
                    nc.sync.dma_start(by[j * 128:(j + 1) * 128, :],
                                      ytm[1][:, j * D:(j + 1) * D])
                artm = pwork.tile([128, NTB * D], BF16, tag="h_fm0", name="artm")
                for j in range(NTB):
                    nc.gpsimd.indirect_dma_start(
                        out=artm[:, j * D:(j + 1) * D], out_offset=None,
                        in_=by[:],
                        in_offset=IndirectOffsetOnAxis(ap=idx[:, j:j + 1], axis=0))
                x_new = perm.tile([128, NTB * D], F32, tag=f"x_tm{(l + 1) % 2}",
                                  name=f"x_tm{(l + 1) % 2}")
                for j in range(NTB):
                    sl = slice(j * D, (j + 1) * D)
                    nc.vector.tensor_tensor(out=x_new[:, sl], in0=x_tm[:, sl],
                                            in1=ytm[0][:, sl], op=OP.add)
                    nc.vector.tensor_tensor(out=x_new[:, sl], in0=x_new[:, sl],
                                            in1=artm[:, sl], op=OP.add)
                x_tm = x_new

            # ---------------- final channel LayerNorm ----------------
            s_t = pwork.tile([128, NTB * D], F32, tag="h_tm", name="s_t")
            nc.gpsimd.tensor_tensor(out=s_t[:], in0=x_tm[:], in1=a_tm[:], op=OP.add)
            stm = psmall.tile([128, NTB], F32, tag="stm", name="stm")
            stv = psmall.tile([128, NTB], F32, tag="stv", name="stv")
            dump = pwork.tile([128, D], F32, tag="sq", name="sq")
            for j in range(NTB):
                nc.scalar.activation(dump[:], s_t[:, j * D:(j + 1) * D], AF.Copy,
                                     accum_out=stm[:, j:j + 1])
                nc.scalar.activation(dump[:], s_t[:, j * D:(j + 1) * D], AF.Square,
                                     accum_out=stv[:, j:j + 1])
            mu = psmall.tile([128, NTB], F32, tag="mu", name="mu")
            nc.vector.tensor_scalar(out=mu[:], in0=stm[:], scalar1=1.0 / D,
                                    scalar2=None, op0=OP.mult)
            var = psmall.tile([128, NTB], F32, tag="var", name="var")
            nc.vector.tensor_scalar(out=var[:], in0=stv[:], scalar1=1.0 / D,
                                    scalar2=None, op0=OP.mult)
            mu2 = psmall.tile([128, NTB], F32, tag="mu2", name="mu2")
            nc.vector.tensor_tensor(out=mu2[:], in0=mu[:], in1=mu[:], op=OP.mult)
            nc.vector.tensor_tensor(out=var[:], in0=var[:], in1=mu2[:],
                                    op=OP.subtract)
            ve = psmall.tile([128, NTB], F32, tag="ve", name="ve")
            nc.vector.tensor_scalar(out=ve[:], in0=var[:], scalar1=EPS,
                                    scalar2=None, op0=OP.add)
            vr = psmall.tile([128, NTB], F32, tag="vr", name="vr")
            nc.vector.reciprocal(out=vr[:], in_=ve[:])
            rstd2 = psmall.tile([128, NTB], F32, tag="rstd2", name="rstd2")
            nc.scalar.activation(rstd2[:], vr[:], AF.Sqrt)
            otm = pwork.tile([128, NTB * D], F32, tag="h_fm0", name="otm")
            for j in range(NTB):
                sl = slice(j * D, (j + 1) * D)
                nc.vector.tensor_scalar(out=otm[:, sl], in0=s_t[:, sl],
                                        scalar1=mu[:, j:j + 1],
                                        scalar2=rstd2[:, j:j + 1],
                                        op0=OP.subtract, op1=OP.mult)
                nc.vector.tensor_tensor(out=otm[:, sl], in0=otm[:, sl],
                                        in1=g_bc[:], op=OP.mult)
                nc.vector.tensor_tensor(out=otm[:, sl], in0=otm[:, sl],
                                        in1=be_bc[:], op=OP.add)
            for j in range(NTB):
                nc.sync.dma_start(
                    out_d[(bi * NTB + j) * 128:(bi * NTB + j + 1) * 128, :],
                    otm[:, j * D:(j + 1) * D])


# ---------------- host side ----------------

def make_in_maps(inputs, num_cores=1):
    inp = {k: np.asarray(v, dtype=np.float32) for k, v in inputs.items()}
    m = {}
    m["a_fm"] = np.ascontiguousarray(inp["audio"].transpose(0, 2, 1))
    m["a_tm"] = np.ascontiguousarray(inp["audio"])
    m["v_fm"] = np.ascontiguousarray(inp["video"].transpose(0, 2, 1))
    m["w_a2v"] = np.ascontiguousarray(inp["gate_a2v_w"].T)
    m["b_a2v"] = np.ascontiguousarray(inp["gate_a2v_b"].reshape(NDCH, 128).T)
    m["w_v2a"] = np.ascontiguousarray(inp["gate_v2a_w"].T)
    m["b_v2a"] = np.ascontiguousarray(inp["gate_v2a_b"].reshape(NDCH, 128).T)
    m["pk"] = np.ascontiguousarray(
        np.stack([inp["proj_w"][:, :, k].T for k in range(3)]))
    m["bn_s"] = np.ascontiguousarray(
        (inp["bn_gamma"] / np.sqrt(1.0 + 1e-5)).reshape(NDCH, 128).T)
    m["bn_b"] = np.ascontiguousarray(inp["bn_beta"].reshape(NDCH, 128).T)
    m["rmsw_bc"] = np.ascontiguousarray(
        np.broadcast_to(inp["rms_w"][:, None, :], (NM, 128, D)))
    m["w_in"] = np.ascontiguousarray(inp["in_proj_w"].transpose(0, 1, 3, 2))
    m["cw"] = np.ascontiguousarray(
        inp["conv_w"].reshape(NM, 2, NICH, 128, DCONV)
        .transpose(0, 1, 3, 2, 4).reshape(NM, 2, 128, NICH * DCONV))
    m["cb"] = np.ascontiguousarray(
        inp["conv_b"].reshape(NM, 2, NICH, 128).transpose(0, 1, 3, 2))
    w_xp_p = np.zeros((NM, 2, DI, 80), np.float32)
    for l in range(NM):
        for r in range(2):
            xp_t = inp["x_proj_w"][l, r].T  # [DI, 48]
            w_xp_p[l, r, :, 0:DTR] = xp_t[:, 0:DTR]
            w_xp_p[l, r, :, 32:32 + DS] = xp_t[:, DTR:DTR + DS]
            w_xp_p[l, r, :, 64:64 + DS] = xp_t[:, DTR + DS:DTR + 2 * DS]
    m["w_xp"] = w_xp_p
    m["w_dt"] = np.ascontiguousarray(inp["dt_w"].transpose(0, 1, 3, 2))
    m["dtb"] = np.ascontiguousarray(
        inp["dt_b"].reshape(NM, 2, NICH, 128).transpose(0, 1, 3, 2))
    m["a_neg"] = np.ascontiguousarray(
        (-np.exp(inp["A_log"])).reshape(NM, 2, NICH, 128, DS)
        .transpose(0, 1, 3, 2, 4).reshape(NM, 2, 128, NICH * DS))
    m["dsk"] = np.ascontiguousarray(
        inp["D_skip"].reshape(NM, 2, NICH, 128).transpose(0, 1, 3, 2))
    m["w_out"] = np.ascontiguousarray(inp["out_w"].transpose(0, 1, 3, 2))
    m["g_bc"] = np.ascontiguousarray(np.broadcast_to(inp["cln_gamma"], (128, D)))
    m["be_bc"] = np.ascontiguousarray(np.broadcast_to(inp["cln_beta"], (128, D)))
    m["id128"] = np.eye(128, dtype=np.float32)
    t_of = np.arange(T, dtype=np.int64).reshape(NTB, 128).T  # [128, NTB]
    m["idx"] = np.ascontiguousarray(((T - 1) - t_of).astype(np.int32))

    parts = []
    for name, shape in PACK:
        a = np.ascontiguousarray(m[name])
        assert a.shape == tuple(shape), (name, a.shape, shape)
        if a.dtype == np.int32:
            a = a.view(np.float32)
        parts.append(a.ravel())
    blob = np.concatenate(parts)
    return [{"blob": blob} for _ in range(num_cores)]


def get_nc(**kw):
    key = ("nc", tuple(sorted(kw.items())))
    if key not in _CACHE:
        _CACHE[key] = build_nc(**kw)
    return _CACHE[key]


def kernel(**inputs) -> np.ndarray:
    nc = get_nc()
    maps = make_in_maps(inputs, 1)
    res = run_bass_kernel_spmd(nc, maps, [0])
    out = res.results[0]["out"].reshape(B, T, D)
    return out.astype(np.float32)


if __name__ == "__main__":
    import reference
    inputs = {k: np.asarray(v) for k, v in reference.setup_inputs().items()}
    got = kernel(**inputs)
    print("kernel ran; out shape", got.shape)
